# revision 2
# baseline (speedup 1.0000x reference)
"""AWQ int8 linear (x @ (W_q * scale).T + bias) on 8 Trainium2 NeuronCores.

Column-parallel sharding: out_features padded 11008 -> 11264 and split 1408
per core; x is replicated. Per core computes outT[o, t] = scale[o] *
sum_i W_q[o, i] * x[t, i] + bias[o] with W_q cast int8->bf16 during the DMA
(exact: |W_q| <= 127 < 256), x pre-cast to bf16 on host, PE accumulating in
fp32 PSUM, and a per-partition scale+bias fused into one ScalarE activation.

Host-side layout prep: x is transposed to xT [IN_F, T] so the contraction
dim lands on SBUF partitions; per-core weights are pre-arranged to the exact
SBUF image [ki=128, m, ko, o] so every DMA is contiguous per partition.
"""

import numpy as np
import ml_dtypes

import concourse.mybir as mybir
import concourse.tile as tile
from concourse import bacc
from concourse.bass_utils import run_bass_kernel_spmd

B, S, IN_F, OUT_F = 4, 512, 4096, 11008
T = B * S                      # 2048
NCORES = 8
O_PAD = 11264                  # next multiple of 8*128 above OUT_F
O_PER = O_PAD // NCORES        # 1408
M_TILES = O_PER // 128         # 11
K_TILES = IN_F // 128          # 32
N_TILE = 512
N_TILES = T // N_TILE          # 4

_NC = None


def _build():
    nc = bacc.Bacc("TRN2", target_bir_lowering=False, debug=False,
                   num_devices=NCORES)
    xT = nc.dram_tensor("xT", [IN_F, T], mybir.dt.bfloat16,
                        kind="ExternalInput")
    wq = nc.dram_tensor("wq", [128, M_TILES * K_TILES * 128], mybir.dt.int8,
                        kind="ExternalInput")
    sb = nc.dram_tensor("sb", [128, 2 * M_TILES], mybir.dt.float32,
                        kind="ExternalInput")
    outT = nc.dram_tensor("outT", [O_PER, T], mybir.dt.float32,
                          kind="ExternalOutput")

    with tile.TileContext(nc) as tc:
        with (
            tc.tile_pool(name="xp", bufs=1) as xp,
            tc.tile_pool(name="wp", bufs=2) as wp,
            tc.tile_pool(name="op", bufs=4) as op,
            tc.tile_pool(name="cp", bufs=1) as cp,
            tc.tile_pool(name="ps", bufs=8, space="PSUM") as pp,
        ):
            # scale in columns [0:M_TILES), bias in [M_TILES:2*M_TILES)
            sb_sb = cp.tile([128, 2 * M_TILES], mybir.dt.float32, tag="sb")
            nc.sync.dma_start(sb_sb[:], sb[:, :])

            # whole xT stays resident in SBUF as bf16 (16.8 MB)
            x_sb = []
            for k in range(K_TILES):
                t = xp.tile([128, T], mybir.dt.bfloat16, tag=f"x{k}")
                nc.sync.dma_start(t[:], xT[k * 128:(k + 1) * 128, :])
                x_sb.append(t)

            for m in range(M_TILES):
                # weights for this m-tile: [ki, ko, o], int8->bf16 in the DMA
                w_sb = wp.tile([128, K_TILES, 128], mybir.dt.bfloat16, tag="w")
                nc.gpsimd.dma_start(
                    w_sb[:],
                    wq[:, m * K_TILES * 128:(m + 1) * K_TILES * 128]
                    .rearrange("p (ko o) -> p ko o", o=128),
                )
                psum = [pp.tile([128, N_TILE], mybir.dt.float32, tag="psum",
                                name=f"psum_{m}_{n}")
                        for n in range(N_TILES)]
                for k in range(K_TILES):
                    for n in range(N_TILES):
                        nc.tensor.matmul(
                            psum[n][:],
                            w_sb[:, k, :],
                            x_sb[k][:, n * N_TILE:(n + 1) * N_TILE],
                            start=(k == 0),
                            stop=(k == K_TILES - 1),
                        )
                for n in range(N_TILES):
                    o_sb = op.tile([128, N_TILE], mybir.dt.float32, tag="o")
                    nc.scalar.activation(
                        o_sb[:], psum[n][:],
                        mybir.ActivationFunctionType.Identity,
                        bias=sb_sb[:, M_TILES + m:M_TILES + m + 1],
                        scale=sb_sb[:, m:m + 1],
                    )
                    nc.sync.dma_start(
                        outT[m * 128:(m + 1) * 128,
                             n * N_TILE:(n + 1) * N_TILE],
                        o_sb[:],
                    )
    nc.compile()
    return nc


def _get_nc():
    global _NC
    if _NC is None:
        _NC = _build()
    return _NC


def _prepare_in_maps(x, W_q, weight_scale, bias):
    x = np.asarray(x, dtype=np.float32)
    W_q = np.asarray(W_q).astype(np.int8, copy=False)
    weight_scale = np.asarray(weight_scale, dtype=np.float32).reshape(-1)
    bias = np.asarray(bias, dtype=np.float32).reshape(-1)

    xT = np.ascontiguousarray(x.reshape(T, IN_F).T).astype(ml_dtypes.bfloat16)

    Wp = np.zeros((O_PAD, IN_F), dtype=np.int8)
    Wp[:OUT_F] = W_q
    scp = np.zeros((O_PAD,), np.float32)
    scp[:OUT_F] = weight_scale
    bsp = np.zeros((O_PAD,), np.float32)
    bsp[:OUT_F] = bias

    # per-core SBUF weight image [ki, m, ko, o]
    Wimg = Wp.reshape(NCORES, M_TILES, 128, K_TILES, 128)  # (c, m, o, ko, ki)
    Wimg = np.ascontiguousarray(Wimg.transpose(0, 4, 1, 3, 2)).reshape(
        NCORES, 128, M_TILES * K_TILES * 128)
    sc = scp.reshape(NCORES, M_TILES, 128).transpose(0, 2, 1)   # (c, p, m)
    bs = bsp.reshape(NCORES, M_TILES, 128).transpose(0, 2, 1)
    sbimg = np.ascontiguousarray(np.concatenate([sc, bs], axis=2),
                                 dtype=np.float32)               # (c, 128, 2M)

    return [{"xT": xT, "wq": Wimg[c], "sb": sbimg[c]} for c in range(NCORES)]


def _gather(results):
    outT = np.concatenate([r["outT"] for r in results], axis=0)  # [O_PAD, T]
    out = outT[:OUT_F].T.reshape(B, S, OUT_F)
    return np.ascontiguousarray(out, dtype=np.float32)


def _run(x, W_q, weight_scale, bias, **run_kwargs):
    nc = _get_nc()
    in_maps = _prepare_in_maps(x, W_q, weight_scale, bias)
    res = run_bass_kernel_spmd(nc, in_maps, core_ids=list(range(NCORES)),
                               **run_kwargs)
    return _gather(res.results), res


def kernel(x, W_q, weight_scale, bias):
    out, _ = _run(x, W_q, weight_scale, bias)
    return out
